# revision 3
# baseline (speedup 1.0000x reference)
"""Trainium2 Bass kernel for nn_Branch_2_36386962932308.

Network (per batch, feature-major planes [channels, L=h*w=4096]):
  stage1: Mamba(d=128, di=128, n=2, r=8, conv4) -> LN
  linear: 128->256 + SiLU   (stage-1 LN affine folded into the linear weight)
  stage2: Mamba(d=256, di=256, n=2, r=16, conv4) -> LN (affine applied on device)

Sharding: data-parallel over batch, one batch element per NeuronCore (8 cores).

Key restructurings (v2 — engine-balanced):
  - Input x[b] is already the feature-major plane [c, h*w]; output likewise.
  - Causal depthwise conv (4 taps) folded into in_proj: 4 shifted matmuls
    accumulated in PSUM (3 leading zero cols provide causal padding).
  - SSM scan via DVE/Pool tensor_tensor_scan chained across 512-col chunks.
  - B/C rows replicated across partitions with selection-matrix matmuls on
    the PE; the PSUM->SBUF rep copies run on the (otherwise idle) GpSimd.
  - LayerNorm mean comes FREE from the out_proj matmul: wout is augmented
    with an extra column w1 = rowsums(wout)/dout, so yp_ps[:, dout] = mu.
    Only sum-of-squares needs an ACT Square+accum pass; (x-mu)*rstd reads
    yp straight from PSUM (no SBUF copy).
  - dA2 = dA1^2 on the DVE (A rows satisfy A2 = 2*A1), saving an ACT Exp.
  - bf16 on all elementwise-heavy paths (xc, dt, dbu, hs, tn, t1n, t2pad,
    out) -> DVE 2x mode; matmul moving operands bf16 -> full PE rate.
  - LN outputs return to feature-major via PE transpose (bf16, 1 cycle/
    row; the XBAR DMA transpose corrupts even lanes on this platform);
    stage-2 LN affine applied by the DVE, output stored bf16, host upcast.
  - ACT table-set discipline: per span one SiLU phase then one
    natural_log_exp phase (softplus = Ln(Exp(x)+1), rstd = Exp(-0.5*Ln(
    var+eps))), with barrier pseudo-ops preventing cross-phase reordering.

Self-contained: hardcodes all shapes; needs only concourse + numpy at runtime.
"""

import os
from contextlib import ExitStack

import numpy as np

import concourse.bass as bass
import concourse.bacc as bacc
import concourse.mybir as mybir
import concourse.tile as tile
from concourse.bass_utils import run_bass_kernel_spmd
from concourse.masks import make_identity

F32 = mybir.dt.float32
BF16 = mybir.dt.bfloat16
AF = mybir.ActivationFunctionType
ALU = mybir.AluOpType

NCORES = 8
LN_EPS = 1e-5
CH = 512          # pipeline column chunk (one PSUM bank at fp32)
SUB = 128         # out_proj / LN subchunk (time-major tile height)
SPAN = 2048       # ACT table-set phase width

last_exec_time_ns = None
last_results = None


def _patch_act_tables():
    """Make natural_log_exp_and_others the only table set containing Exp and
    Ln, so bacc's table-load placement keeps one set resident through the
    whole post-SiLU phase instead of swapping between exp_and_others and
    natural_log on every Exp<->Ln transition (~2.7us per swap). Set ids and
    ordering are preserved; only membership is filtered."""
    import functools
    import concourse.hw_specs as hw_specs
    if getattr(hw_specs.get_activation_tables, "_lnexp_patched", False):
        return
    orig = hw_specs.get_activation_tables

    @functools.cache
    def patched(arch):
        tables = {k: set(v) for k, v in orig(arch).items()}
        for name, fns in tables.items():
            if name != 'natural_log_exp_and_others':
                fns.discard(AF.Exp)
                fns.discard(AF.Ln)
        return tables

    patched._lnexp_patched = True
    hw_specs.get_activation_tables = patched
    bacc.get_activation_tables = patched


_patch_act_tables()


# ----------------------------------------------------------------------------
# host-side weight preparation
# ----------------------------------------------------------------------------

def _prep_stage(p, d, di, r):
    win = np.asarray(p['win'], np.float32)
    b_in = np.asarray(p['bin'], np.float32)
    cw = np.asarray(p['cw'], np.float32)        # [di, 1, 4]
    cb = np.asarray(p['cb'], np.float32)
    wx = np.asarray(p['wx'], np.float32)        # [r+4, di]
    wdt = np.asarray(p['wdt'], np.float32)      # [di, r]
    bdt = np.asarray(p['bdt'], np.float32)
    alog = np.asarray(p['alog'], np.float32)    # [di, 2]
    dd = np.asarray(p['dd'], np.float32)
    wout = np.asarray(p['wout'], np.float32)    # [dout, di]

    winx, winz = win[:di], win[di:]
    w_k = np.stack([np.ascontiguousarray((cw[:, 0, k:k + 1] * winx).T)
                    for k in range(4)])          # [4, d, di]
    wz = np.ascontiguousarray(winz.T)            # [d, di]
    wxT = np.ascontiguousarray(wx.T)             # [di, r+4]
    wdtT = np.ascontiguousarray(wdt.T)           # [r, di]
    dout = wout.shape[0]
    woutT = np.ascontiguousarray(wout.T)         # [di, dout]
    w1 = woutT.sum(1, keepdims=True) / dout      # mean column -> mu for free
    zpad = np.zeros_like(w1)                     # pad to even N for the ISA
    woutA = np.ascontiguousarray(
        np.concatenate([woutT, w1, zpad], 1))    # [di, dout+2]

    S = cw[:, 0, :].sum(1)
    silu_bias = cb + S * b_in[:di]
    bz = b_in[di:]
    A = -np.exp(alog)                            # [di, 2] (negative)
    corr = np.stack([-(cw[:, 0, :3 - t].sum(1)) * b_in[:di] for t in range(3)], 1)
    cols = [silu_bias, bz, bdt, A[:, 0], A[:, 1], dd,
            corr[:, 0], corr[:, 1], corr[:, 2]]
    sel = np.zeros((4, r + 4, 128), np.float32)
    for j in range(4):
        sel[j, r + j, :] = 1.0
    return w_k, wz, wxT, wdtT, woutA, np.stack(cols, 1).astype(np.float32), sel


def prep_weights(inputs):
    s1 = {k[3:]: inputs[k] for k in inputs if k.startswith('s1_')}
    s2 = {k[3:]: inputs[k] for k in inputs if k.startswith('s2_')}
    w1k, w1z, wx1, wdt1, wout1, cols1, sel1 = _prep_stage(s1, 128, 128, 8)
    w2k, w2z, wx2, wdt2, wout2, cols2, sel2 = _prep_stage(s2, 256, 256, 16)
    lnw2 = np.asarray(s2['lnw'], np.float32)
    lnb2 = np.asarray(s2['lnb'], np.float32)
    cols2 = np.concatenate([cols2, lnw2[:, None], lnb2[:, None]], 1)
    cols2 = np.ascontiguousarray(cols2, dtype=np.float32)

    bfdt = mybir.dt.np(BF16)
    lin_w = np.asarray(inputs['lin_w'], np.float32)
    lin_b = np.asarray(inputs['lin_b'], np.float32)
    lnw1 = np.asarray(s1['lnw'], np.float32)
    lnb1 = np.asarray(s1['lnb'], np.float32)
    linw = np.ascontiguousarray((lin_w * lnw1[None, :]).T)
    linb = (lin_w @ lnb1 + lin_b).astype(np.float32)[:, None]

    return {
        'idenb': np.eye(128, dtype=np.float32).astype(bfdt),
        'w1k': w1k, 'w1z': w1z, 'wx1': wx1.astype(bfdt),
        'wdt1': wdt1.astype(bfdt),
        'wout1': wout1.astype(bfdt), 'cols1': cols1,
        'w2k': w2k.astype(bfdt), 'w2z': w2z.astype(bfdt),
        'wx2': wx2.astype(bfdt), 'wdt2': wdt2.astype(bfdt),
        'wout2': wout2.astype(bfdt), 'cols2': cols2,
        'linw': linw.astype(bfdt), 'linb': linb,
    }


# ----------------------------------------------------------------------------
# device program
# ----------------------------------------------------------------------------

F32R = mybir.dt.float32r


def _tile(pool, shape, dtype, tag, bufs=None):
    return pool.tile(shape, dtype, tag=tag, name=tag, bufs=bufs)


def _mmr(nc, out, lhsT, rhs, **kw):
    """fp32 matmul via float32r bitcast: single-pass on the PE (fp32 proper
    runs as two half-speed passes)."""
    nc.tensor.matmul(out, lhsT.bitcast(F32R), rhs.bitcast(F32R), **kw)


def _mmb(nc, out, lhsT, rhs, **kw):
    """all-bf16 matmul (full PE rate at any N)."""
    nc.tensor.matmul(out, lhsT, rhs, **kw)


class _ActChain:
    """Groups ACT instructions into table-set phases separated by no-op
    barrier instructions, so the scheduler can reorder freely within a phase
    (same table set) but cannot interleave phases (which would make bacc
    insert a ~2.7us ACT table load per out-of-phase function switch)."""

    def __init__(self, nc, bar_tile):
        self.nc = nc
        self.bar_tile = bar_tile
        self.group = []
        self.barrier = None

    def new_group(self):
        from concourse.tile_rust import add_dep_helper
        if not self.group:
            return
        bar = self.nc.scalar.activation(self.bar_tile[:], self.bar_tile[:],
                                        AF.Identity)
        barc = bar.ins if hasattr(bar, 'ins') else bar
        for op in self.group:
            add_dep_helper(barc, op, sync=False, reason="act phase barrier")
        self.barrier = barc
        self.group = []

    def __call__(self, *args, **kwargs):
        from concourse.tile_rust import add_dep_helper
        inst = self.nc.scalar.activation(*args, **kwargs)
        cur = inst.ins if hasattr(inst, 'ins') else inst
        if self.barrier is not None:
            add_dep_helper(cur, self.barrier, sync=False,
                           reason="act phase order")
        self.group.append(cur)
        return inst


def _stage_phase_a(nc, pools, cfg, s0):
    """in_proj (conv-folded) + z + SiLU for one span -> xc/sz span planes."""
    ps_mm = pools['mm']
    P_in, P = cfg['P_in'], cfg['P']
    planes, wk, wz, cols = (cfg['in_planes'], cfg['wk_sb'], cfg['wz_sb'],
                            cfg['cols_sb'])
    mmin = cfg['mm_in']
    xc_sp, sz_sp = cfg['xc_sp'], cfg['sz_sp']
    for ci in range(SPAN // CH):
        c0 = s0 + ci * CH
        lc = ci * CH
        for mi in range(P):
            ms = slice(mi * 128, (mi + 1) * 128)
            xc_ps = _tile(ps_mm, [128, CH], F32, "mm", 3)
            nmm = 4 * P_in
            i = 0
            for k in range(4):
                for kt in range(P_in):
                    mmin(nc, xc_ps[:], wk[k][kt][:, ms],
                         planes[kt][:, c0 + k: c0 + k + CH],
                         start=(i == 0), stop=(i == nmm - 1))
                    i += 1
            if c0 == 0:
                nc.vector.tensor_add(xc_ps[:, 0:3], xc_ps[:, 0:3],
                                     cols[mi][:, 6:9])
            cfg['act'](xc_sp[mi][:, lc:lc + CH], xc_ps[:], AF.Silu,
                       bias=cols[mi][:, 0:1])

            z_ps = _tile(ps_mm, [128, CH], F32, "mm", 3)
            for kt in range(P_in):
                mmin(nc, z_ps[:], wz[kt][:, ms],
                     planes[kt][:, c0 + 3: c0 + 3 + CH],
                     start=(kt == 0), stop=(kt == P_in - 1))
            cfg['act'](sz_sp[mi][:, lc:lc + CH], z_ps[:], AF.Silu,
                       bias=cols[mi][:, 1:2])


MAGIC = 0x5f3759df


def _stage_phase_b(nc, pools, cfg, s0, hs_prev):
    """Everything after SiLU for one span. ACT-chained ops (dt chain) are all
    early; Squares/normalize use table-free functions so they never force an
    ACT table load and carry no phase ordering."""
    sb = pools['sb']
    ps_mm, ps_o = pools['mm'], pools['o']
    P_in, P, r, dout = cfg['P_in'], cfg['P'], cfg['r'], cfg['dout']
    rw = r + 4
    wxs, wdts, wouts, cols = (cfg['wx_sb'], cfg['wdt_sb'], cfg['wout_sb'],
                              cfg['cols_sb'])
    xc_sp, sz_sp = cfg['xc_sp'], cfg['sz_sp']
    scan = cfg['scan_eng'].tensor_tensor_scan
    a2_is_2a1 = cfg['a2_is_2a1']
    actf = nc.scalar.activation          # table-free ACT ops (any table set)

    for ci in range(SPAN // CH):
        c0 = s0 + ci * CH
        lc = ci * CH
        lcs = slice(lc, lc + CH)
        # ---- wx projection -> dtin rows + B/C rows ----
        xdbl_ps = _tile(ps_mm, [128, CH], F32, "mm", 3)
        for kt in range(P_in):
            _mmb(nc, xdbl_ps[:rw, :], wxs[kt][:], xc_sp[kt][:, lcs],
                 start=(kt == 0), stop=(kt == P_in - 1))
        xdbl = _tile(sb, [rw, CH], BF16, "xdbl", 2)
        actf(xdbl[:], xdbl_ps[:rw, :], AF.Identity)

        # ---- dt = softplus = Ln(Exp(raw + bdt) + 1); dA_n = exp(A_n*dt) ----
        dt_sb = []
        dA_sb = [[None] * P, [None] * P]
        for mi in range(P):
            ms = slice(mi * 128, (mi + 1) * 128)
            dt_ps = _tile(ps_mm, [128, CH], F32, "mm", 3)
            _mmb(nc, dt_ps[:], wdts[:, ms], xdbl[:r, :])
            t_e = _tile(sb, [128, CH], F32, "scr1", 3)
            cfg['act'](t_e[:], dt_ps[:], AF.Exp, bias=cols[mi][:, 2:3])
            t_dt = _tile(sb, [128, CH], BF16, "dt", 3)
            cfg['act'](t_dt[:], t_e[:], AF.Ln, bias=1.0)
            dt_sb.append(t_dt)
            t_dA0 = _tile(sb, [128, CH], F32, "dA0", 3)
            cfg['act'](t_dA0[:], t_dt[:], AF.Exp, scale=cols[mi][:, 3:4])
            dA_sb[0][mi] = t_dA0
            t_dA1 = _tile(sb, [128, CH], F32, "dA1", 3)
            if a2_is_2a1:
                nc.gpsimd.tensor_mul(t_dA1[:], t_dA0[:], t_dA0[:])
            else:
                cfg['act'](t_dA1[:], t_dt[:], AF.Exp, scale=cols[mi][:, 4:5])
            dA_sb[1][mi] = t_dA1

        # ---- replicate B/C rows across partitions: DMA-flatten the four
        # xdbl rows to one partition, then one Pool partition_broadcast ----
        rows = _tile(sb, [1, 4 * CH], BF16, "rows", 2)
        nc.scalar.dma_start(rows[:], xdbl[r:r + 4, :])
        rep_all = _tile(sb, [128, 4 * CH], BF16, "repall", 2)
        nc.gpsimd.partition_broadcast(rep_all[:], rows[:])
        rep = [rep_all[:, j * CH:(j + 1) * CH] for j in range(4)]
        hs = [None] * P                      # [128, 2*CH]: n=0 | n=1 halves
        for mi in range(P):
            t_u = _tile(sb, [128, CH], BF16, "scr2", 3)
            nc.vector.tensor_mul(t_u[:], dt_sb[mi][:], xc_sp[mi][:, lcs])
            t_hs = _tile(sb, [128, 2 * CH], BF16, "hs", 3)
            for n in range(2):
                t_dbu = _tile(sb, [128, CH], BF16, f"dbu{n}", 2)
                nc.vector.tensor_mul(t_dbu[:], t_u[:], rep[n])
                init = 0.0 if c0 == 0 else hs_prev[n][mi]
                scan(t_hs[:, n * CH:(n + 1) * CH], dA_sb[n][mi][:],
                     t_dbu[:], init, ALU.mult, ALU.add)
                hs_prev[n][mi] = t_hs[:, (n + 1) * CH - 1:(n + 1) * CH]
            hs[mi] = t_hs

        # ---- y = (hs0*C0 + hs1*C1 + dd*xc) * silu(z) ----
        yg_sb = []
        for mi in range(P):
            t_m = _tile(sb, [128, 2 * CH], BF16, "m2", 2)
            nc.vector.tensor_mul(t_m[:], hs[mi][:], rep_all[:, 2 * CH:])
            t_y = _tile(sb, [128, CH], BF16, "y", 2)
            nc.vector.tensor_add(t_y[:], t_m[:, :CH], t_m[:, CH:])
            t_dx = _tile(sb, [128, CH], BF16, "dx", 2)
            nc.vector.tensor_scalar(t_dx[:], xc_sp[mi][:, lcs],
                                    cols[mi][:, 5:6], None,
                                    ALU.mult, ALU.bypass)
            nc.vector.tensor_add(t_y[:], t_y[:], t_dx[:])
            t_yg = _tile(sb, [128, CH], BF16, "yg", 3)
            nc.vector.tensor_mul(t_yg[:], t_y[:], sz_sp[mi][:, lcs])
            yg_sb.append(t_yg)

        # ---- out_proj (time-major, mu via augmented column) + LayerNorm;
        # rstd = 1/sqrt(var+eps) via bit-trick seed (DVE int ops) + one
        # Newton step (Pool), so no Ln/Exp table ops gate the span tail ----
        d1 = dout + 2
        mu4 = _tile(sb, [SUB, 4], F32, "mu4")
        ssq4 = _tile(sb, [SUB, 4], F32, "ssq4")
        yp_all = []
        for g in range(4):
            cs = slice(g * SUB, (g + 1) * SUB)
            yp_ps = _tile(ps_o, [SUB, 512], F32, "yp", 4)
            for mi in range(P):
                nc.tensor.matmul(yp_ps[:, :d1], yg_sb[mi][:, cs],
                                 wouts[mi][:],
                                 start=(mi == 0), stop=(mi == P - 1))
            scr = _tile(sb, [SUB, dout], BF16, "scr", 2)
            actf(scr[:], yp_ps[:, :dout], AF.Square,
                 accum_out=ssq4[:, g:g + 1])
            nc.vector.tensor_scalar(mu4[:, g:g + 1],
                                    yp_ps[:, dout:dout + 1],
                                    1.0, None, ALU.mult, ALU.bypass)
            yp_all.append(yp_ps)
        # veps = ssq/dout - mu^2 + eps   (Pool)
        musq = _tile(sb, [SUB, 4], F32, "musq")
        nc.gpsimd.tensor_mul(musq[:], mu4[:], mu4[:])
        veps = _tile(sb, [SUB, 4], F32, "veps")
        nc.gpsimd.tensor_scalar(veps[:], ssq4[:], 1.0 / dout, LN_EPS,
                                ALU.mult, ALU.add)
        nc.gpsimd.tensor_sub(veps[:], veps[:], musq[:])
        # bit-trick rsqrt seed (DVE int ALU)
        i1 = _tile(sb, [SUB, 4], mybir.dt.int32, "i1")
        nc.vector.tensor_scalar(i1[:], veps[:].bitcast(mybir.dt.int32),
                                1, None, ALU.arith_shift_right, ALU.bypass)
        nc.vector.tensor_scalar(i1[:], i1[:], 0xFFFFFFFF, None,
                                ALU.bitwise_xor, ALU.bypass)
        nc.vector.tensor_scalar(i1[:], i1[:], MAGIC + 1, None,
                                ALU.add, ALU.bypass)
        r0 = i1[:].bitcast(F32)
        # two Newton steps (Pool): r <- r*(1.5 - 0.5*veps*r^2)
        vh = _tile(sb, [SUB, 4], F32, "vh")
        nc.gpsimd.tensor_scalar(vh[:], veps[:], -0.5, None,
                                ALU.mult, ALU.bypass)
        for it in range(2):
            t_nr = _tile(sb, [SUB, 4], F32, f"nr{it}")
            nc.gpsimd.tensor_mul(t_nr[:], r0, vh[:])
            nc.gpsimd.tensor_mul(t_nr[:], t_nr[:], r0)
            nc.gpsimd.tensor_scalar(t_nr[:], t_nr[:], 1.5, None,
                                    ALU.add, ALU.bypass)
            r1 = _tile(sb, [SUB, 4], F32, f"rst{it}")
            nc.gpsimd.tensor_mul(r1[:], r0, t_nr[:])
            r0 = r1[:]
        rstd4 = r0
        # nmr = -mu * rstd  (normalize bias)
        nmr = _tile(sb, [SUB, 4], F32, "nmr")
        nc.gpsimd.tensor_scalar(nmr[:], mu4[:], -1.0, None,
                                ALU.mult, ALU.bypass)
        nc.gpsimd.tensor_mul(nmr[:], nmr[:], rstd4)
        for g in range(4):
            tn = _tile(sb, [SUB, dout], BF16, "tn", 4)
            actf(tn[:], yp_all[g][:, :dout], AF.Identity,
                 scale=rstd4[:, g:g + 1], bias=nmr[:, g:g + 1])
            cfg['emit'](tn, c0, g)
        cfg['flush'](c0)


def build_program(L=4096, a2_is_2a1=True, debug=False):
    nc = bacc.Bacc()
    dp = nc.declare_dram_parameter
    x_d = dp("x", [128, L], F32R, isOutput=False)
    w1k_d = dp("w1k", [4, 128, 128], F32R, isOutput=False)
    w1z_d = dp("w1z", [128, 128], F32R, isOutput=False)
    wx1_d = dp("wx1", [128, 12], BF16, isOutput=False)
    wdt1_d = dp("wdt1", [8, 128], BF16, isOutput=False)
    wout1_d = dp("wout1", [128, 130], BF16, isOutput=False)
    cols1_d = dp("cols1", [128, 9], F32, isOutput=False)
    w2k_d = dp("w2k", [4, 256, 256], BF16, isOutput=False)
    w2z_d = dp("w2z", [256, 256], BF16, isOutput=False)
    wx2_d = dp("wx2", [256, 20], BF16, isOutput=False)
    wdt2_d = dp("wdt2", [16, 256], BF16, isOutput=False)
    wout2_d = dp("wout2", [256, 258], BF16, isOutput=False)
    cols2_d = dp("cols2", [256, 11], F32, isOutput=False)
    linw_d = dp("linw", [128, 256], BF16, isOutput=False)
    linb_d = dp("linb", [256, 1], F32, isOutput=False)
    iden_d = dp("idenb", [128, 128], BF16, isOutput=False)
    out_d = dp("out", [256, L], BF16, isOutput=True)
    if debug:
        dbg_t1n_d = dp("dbg_t1n", [128, L], BF16, isOutput=True)
        dbg_t2_d = dp("dbg_t2", [256, L], BF16, isOutput=True)

    dma = nc.sync.dma_start

    with tile.TileContext(nc) as tc, ExitStack() as ctx:
        consts = ctx.enter_context(tc.tile_pool(name="consts", bufs=1))
        planes = ctx.enter_context(tc.tile_pool(name="planes", bufs=1))
        spans = ctx.enter_context(tc.tile_pool(name="spans", bufs=1))
        sb = ctx.enter_context(tc.tile_pool(name="sb", bufs=2))
        ps_mm = ctx.enter_context(
            tc.tile_pool(name="psmm", bufs=2, space=bass.MemorySpace.PSUM))
        ps_o = ctx.enter_context(
            tc.tile_pool(name="pso", bufs=1, space=bass.MemorySpace.PSUM))
        pools = {'sb': sb, 'mm': ps_mm, 'o': ps_o}

        xpad = planes.tile([128, L + 3], F32R, tag="xpad", name="xpad")
        nc.gpsimd.memset(xpad[:, 0:3].bitcast(F32), 0.0)
        for s0 in range(0, L, SPAN):
            dma(xpad[:, 3 + s0: 3 + s0 + SPAN], x_d[:, s0:s0 + SPAN])

        _ld = [0]

        def load(dram_ap, shape, dtype=F32R):
            _ld[0] += 1
            t = consts.tile(shape, dtype, tag=f"w{_ld[0]}", name=f"w{_ld[0]}")
            dma(t[:], dram_ap)
            return t

        w1k_sb = [[load(w1k_d[k], [128, 128])] for k in range(4)]
        w1z_sb = [load(w1z_d[:], [128, 128])]
        wx1_sb = [load(wx1_d[:], [128, 12], BF16)]
        wdt1_sb = load(wdt1_d[:], [8, 128], BF16)
        wout1_sb = [load(wout1_d[:], [128, 130], BF16)]
        cols1_sb = [load(cols1_d[:], [128, 9], F32)]
        w2k_sb = [[load(w2k_d[k, kt * 128:(kt + 1) * 128], [128, 256], BF16)
                   for kt in range(2)] for k in range(4)]
        w2z_sb = [load(w2z_d[kt * 128:(kt + 1) * 128], [128, 256], BF16)
                  for kt in range(2)]
        wx2_sb = [load(wx2_d[kt * 128:(kt + 1) * 128], [128, 20], BF16)
                  for kt in range(2)]
        wdt2_sb = load(wdt2_d[:], [16, 256], BF16)
        wout2_sb = [load(wout2_d[kt * 128:(kt + 1) * 128], [128, 258], BF16)
                    for kt in range(2)]
        cols2_sb = [load(cols2_d[kt * 128:(kt + 1) * 128], [128, 11], F32)
                    for kt in range(2)]
        linw_sb = load(linw_d[:], [128, 256], BF16)
        linb_sb = [load(linb_d[kt * 128:(kt + 1) * 128], [128, 1], F32)
                   for kt in range(2)]

        bar_tile = consts.tile([1, 1], F32, tag="actbar", name="actbar")
        nc.gpsimd.memset(bar_tile[:], 0.0)
        epsc = consts.tile([128, 1], F32, tag="epsc", name="epsc")
        nc.gpsimd.memset(epsc[:], LN_EPS)
        act_chain = _ActChain(nc, bar_tile)
        ident = consts.tile([128, 128], BF16, tag="ident", name="ident")
        dma(ident[:], iden_d[:])

        t1n = planes.tile([128, L], BF16, tag="t1n", name="t1n")
        t2pad = [planes.tile([128, L + 3], BF16, tag=f"t2pad{mi}",
                             name=f"t2pad{mi}") for mi in range(2)]
        for mi in range(2):
            nc.gpsimd.memset(t2pad[mi][:, 0:3], 0.0)

        def span_tiles(P):
            xc_sp = [_tile(spans, [128, SPAN], BF16, f"xcsp{mi}", 2)
                     for mi in range(P)]
            sz_sp = [_tile(spans, [128, SPAN], BF16, f"szsp{mi}", 2)
                     for mi in range(P)]
            return xc_sp, sz_sp

        # ---- stage 1 ----
        def emit1(tn, c0, g):
            tf = _tile(ps_o, [128, 2 * SUB], BF16, "tf", 1)
            nc.tensor.transpose(tf[:, :SUB], tn[:], ident[:])
            nc.vector.tensor_scalar(
                t1n[:, c0 + g * SUB: c0 + (g + 1) * SUB], tf[:, :SUB],
                1.0, None, ALU.mult, ALU.bypass)

        cfg1 = dict(
            L=L, P_in=1, P=1, r=8, dout=128, in_planes=[xpad],
            wk_sb=w1k_sb, wz_sb=w1z_sb, wx_sb=wx1_sb, wdt_sb=wdt1_sb,
            wout_sb=wout1_sb, cols_sb=cols1_sb,
            emit=emit1, flush=lambda c0: None, act=act_chain, epsc=epsc,
            a2_is_2a1=a2_is_2a1, scan_eng=nc.vector, mm_in=_mmr)
        hs_prev1 = [[None], [None]]
        for s0 in range(0, L, SPAN):
            xc_sp, sz_sp = span_tiles(1)
            cfg1['xc_sp'], cfg1['sz_sp'] = xc_sp, sz_sp
            act_chain.new_group()
            _stage_phase_a(nc, pools, cfg1, s0)
            act_chain.new_group()
            _stage_phase_b(nc, pools, cfg1, s0, hs_prev1)

        # ---- stage 2 (the linear+SiLU joins each span's SiLU phase) ----
        of_buf = [None, None]

        def emit2(tn, c0, g):
            tf = _tile(ps_o, [128, 2 * SUB], BF16, "tf", 1)
            for ct in range(2):
                if g == 0:
                    of_buf[ct] = _tile(sb, [128, CH], BF16, f"of{ct}", 2)
                nc.tensor.transpose(tf[:, ct * SUB:(ct + 1) * SUB],
                                    tn[:, ct * 128:(ct + 1) * 128],
                                    ident[:])
                nc.vector.tensor_scalar(
                    of_buf[ct][:, g * SUB:(g + 1) * SUB],
                    tf[:, ct * SUB:(ct + 1) * SUB],
                    cols2_sb[ct][:, 9:10], cols2_sb[ct][:, 10:11],
                    ALU.mult, ALU.add)

        def flush2(c0):
            for ct in range(2):
                dma(out_d[ct * 128:(ct + 1) * 128, c0:c0 + CH],
                    of_buf[ct][:])

        cfg2 = dict(
            L=L, P_in=2, P=2, r=16, dout=256, in_planes=t2pad,
            wk_sb=w2k_sb, wz_sb=w2z_sb, wx_sb=wx2_sb, wdt_sb=wdt2_sb,
            wout_sb=wout2_sb, cols_sb=cols2_sb,
            emit=emit2, flush=flush2, act=act_chain, epsc=epsc,
            a2_is_2a1=a2_is_2a1, scan_eng=nc.vector, mm_in=_mmb)
        hs_prev2 = [[None, None], [None, None]]
        for s0 in range(0, L, SPAN):
            # linear + silu for this span (same SiLU table set as phase A)
            act_chain.new_group()
            for c0 in range(s0, s0 + SPAN, CH):
                for mi in range(2):
                    ms = slice(mi * 128, (mi + 1) * 128)
                    lp = _tile(ps_mm, [128, CH], F32, "mm", 3)
                    _mmb(nc, lp[:], linw_sb[:, ms], t1n[:, c0:c0 + CH])
                    act_chain(t2pad[mi][:, 3 + c0: 3 + c0 + CH],
                              lp[:], AF.Silu,
                              bias=linb_sb[mi][:, 0:1])
            xc_sp, sz_sp = span_tiles(2)
            cfg2['xc_sp'], cfg2['sz_sp'] = xc_sp, sz_sp
            _stage_phase_a(nc, pools, cfg2, s0)
            act_chain.new_group()
            _stage_phase_b(nc, pools, cfg2, s0, hs_prev2)

        if debug:
            dma(dbg_t1n_d[:], t1n[:])
            for mi in range(2):
                dma(dbg_t2_d[mi * 128:(mi + 1) * 128, :],
                    t2pad[mi][:, 3:])

    nc.finalize()
    return nc


# ----------------------------------------------------------------------------
# entry point
# ----------------------------------------------------------------------------

_NC = {}


def kernel(**inputs):
    global last_exec_time_ns, last_results
    inputs = {k: np.asarray(v) for k, v in inputs.items()}
    weights = prep_weights(inputs)
    x = inputs['x'].astype(np.float32)          # [8, 128, 64, 64]
    b, c, h, w = x.shape
    L = h * w

    a1 = -np.exp(np.asarray(inputs['s1_alog'], np.float32))
    a2 = -np.exp(np.asarray(inputs['s2_alog'], np.float32))
    a2_is_2a1 = (np.allclose(a1[:, 1], 2 * a1[:, 0], rtol=1e-6) and
                 np.allclose(a2[:, 1], 2 * a2[:, 0], rtol=1e-6))
    key = (L, a2_is_2a1)
    if key not in _NC:
        _NC[key] = build_program(L, a2_is_2a1)

    in_maps = [dict(weights, x=np.ascontiguousarray(x[i].reshape(c, L)))
               for i in range(NCORES)]
    res = run_bass_kernel_spmd(
        _NC[key], in_maps, list(range(NCORES)),
        trace=bool(os.environ.get("KBENCH_TRACE")))
    last_exec_time_ns = res.exec_time_ns
    last_results = res
    out = np.stack([np.asarray(res.results[i]['out'], np.float32)
                    .reshape(256, h, w) for i in range(NCORES)])
    return out

